# revision 6
# baseline (speedup 1.0000x reference)
"""Trainium2 Bass kernel for nn_LocalLinkage (3x LocallyConnected1D, K=S=2, C=F=1).

Math: the three locally-connected layers with unshared weights and
stride==kernel_size form a disjoint 8-leaf weighted reduction tree per
output position:

    out[b, p] = sum_{i<8} E[8p+i] * x[b, 8p+i] + Beff[p]

with E the per-leaf product of the three layer weights along the path and
Beff the folded bias.  E/Beff are computed ON DEVICE once per core (cheap),
then each batch row is one elementwise multiply + grouped sum-of-8.

Sharding: data-parallel over batch, 8 cores x 32 batches.  Each core reads
its x slice (32MB), the full (tiny) weights, writes its out slice (4MB).
"""

import numpy as np

import concourse.bass as bass
import concourse.mybir as mybir
import concourse.tile as tile
from concourse import bass_utils

F32 = mybir.dt.float32

B = 256
L = 262144
N_CORES = 8
B_PER = B // N_CORES          # 32 batches per core
P_OUT = L // 8                # 32768 output positions
XF = L // 128                 # 2048 x elems per partition
OF = P_OUT // 128             # 256 out elems per partition

# Module-level knobs test.py may flip (harness uses defaults).
TRACE = False
LAST_RESULT = None
USE_SCAN = True  # fused multiply+cumsum custom DVE op vs stock mul+reduce


def _register_mul_cumsum():
    """Custom DVE op: out = cumsum(in0 * in1) along the free dim, fp32.

    One 1x-rate pass replaces tensor_mul + grouped tensor_reduce; segment
    sums of 8 are recovered as differences of the cumsum at segment ends.
    """
    import concourse.dve_ops as dve_ops
    from concourse.dve_spec import Spec, Src0, Src1, scan, lower
    from concourse.dve_uop import AluOp, DveOpSpec

    name = "MUL_CUMSUM_LL"
    for o in dve_ops.OPS:
        if o.name == name:
            return o
    spec = Spec(
        body=scan(AluOp.ADD, Src0 * Src1),
        reference=lambda in0, in1, s0, s1, imm2: np.cumsum(
            in0.astype(np.float32) * in1.astype(np.float32), axis=-1, dtype=np.float32
        ),
    )
    row = dve_ops._CUSTOM_DVE_ROW_BASE + len(dve_ops.OPS)
    shas = {}
    for ver in ("v3", "v4"):
        s = DveOpSpec(name=name, opcode=row, uops=lower(spec, ver=ver), rd1_en=True)
        shas[ver] = s.sha(ver)
    op = dve_ops.DveOp(name, spec, subdim=False, uops_sha=shas)
    dve_ops.OPS.append(op)
    dve_ops._SUB_OPCODE_FOR_NAME[name] = row
    dve_ops.CUSTOM_DVE_SPECS[name] = spec
    return op


def _build(b_per=B_PER):
    nc = bass.Bass("TRN2", target_bir_lowering=False, debug=False)

    x = nc.dram_tensor("x", [b_per, L], F32, kind="ExternalInput").ap()
    w0 = nc.dram_tensor("w0", [2 * (L // 2)], F32, kind="ExternalInput").ap()
    b0 = nc.dram_tensor("b0", [L // 2], F32, kind="ExternalInput").ap()
    w1 = nc.dram_tensor("w1", [2 * (L // 4)], F32, kind="ExternalInput").ap()
    w2 = nc.dram_tensor("w2", [2 * (L // 8)], F32, kind="ExternalInput").ap()
    out = nc.dram_tensor("out", [b_per, P_OUT], F32, kind="ExternalOutput").ap()

    ADD = mybir.AluOpType.add
    X = mybir.AxisListType.X

    with tile.TileContext(nc) as tc:
        with (
            tc.tile_pool(name="consts", bufs=1) as consts,
            tc.tile_pool(name="xin", bufs=4) as xpool,
            tc.tile_pool(name="prod", bufs=2) as ppool,
            tc.tile_pool(name="red", bufs=2) as rpool,
            tc.tile_pool(name="outp", bufs=4) as opool,
        ):
            # ---- load weights (layouts line up per partition q):
            #  w0t[q, 2*j0+k0] = W0[q*1024 + j0, k0]
            #  b0t[q, j0]      = b0[q*1024 + j0]
            #  w1t[q, 2*j1+k1] = W1[q*512 + j1, k1]
            #  w2t[q, 2*j2+k2] = W2[q*256 + j2, k2]
            w0t = consts.tile([128, 2048], F32)
            nc.sync.dma_start(w0t[:], w0.rearrange("(p m) -> p m", p=128))
            b0t = consts.tile([128, 1024], F32)
            nc.sync.dma_start(b0t[:], b0.rearrange("(p m) -> p m", p=128))
            w1t = consts.tile([128, 1024], F32)
            nc.sync.dma_start(w1t[:], w1.rearrange("(p m) -> p m", p=128))
            w2t = consts.tile([128, 512], F32)
            nc.sync.dma_start(w2t[:], w2.rearrange("(p m) -> p m", p=128))

            # ---- fold layers: C[4j2+2k2+k1] = W2[j2,k2]*W1[2j2+k2,k1]
            # (route w2t through a same-engine copy first: walrus allows only
            # one semaphore wait on a compute instruction, and ct's mul would
            # otherwise wait on two DMA-lane semaphores)
            w2x = consts.tile([128, 512], F32)
            nc.vector.tensor_copy(w2x[:], w2t[:])
            ct = consts.tile([128, 1024], F32)
            nc.vector.tensor_mul(
                ct[:].rearrange("p (a b) -> p a b", b=2),
                w2x[:].unsqueeze(2).broadcast_to([128, 512, 2]),
                w1t[:].rearrange("p (a b) -> p a b", b=2),
            )
            # E[8j2+4k2+2k1+k0] = C[...]*W0[4j2+2k2+k1, k0]
            et = consts.tile([128, 2048], F32)
            nc.vector.tensor_mul(
                et[:].rearrange("p (a b) -> p a b", b=2),
                ct[:].unsqueeze(2).broadcast_to([128, 1024, 2]),
                w0t[:].rearrange("p (a b) -> p a b", b=2),
            )
            # Beff[j2] = sum_{k2,k1} C[4j2+2k2+k1] * b0[4j2+2k2+k1]
            tt = consts.tile([128, 1024], F32)
            nc.vector.tensor_mul(tt[:], ct[:], b0t[:])
            befft = consts.tile([128, OF], F32)
            nc.vector.tensor_reduce(
                befft[:], tt[:].rearrange("p (a b) -> p a b", b=4), axis=X, op=ADD
            )

            # ---- batch loop
            for b in range(b_per):
                xt = xpool.tile([128, XF], F32)
                nc.sync.dma_start(xt[:], x[b].rearrange("(p m) -> p m", p=128))

                prod = ppool.tile([128, XF], F32)
                nc.vector.tensor_mul(prod[:], xt[:], et[:])

                red = rpool.tile([128, OF], F32)
                nc.vector.tensor_reduce(
                    red[:], prod[:].rearrange("p (a b) -> p a b", b=8), axis=X, op=ADD
                )

                outt = opool.tile([128, OF], F32)
                nc.vector.tensor_add(outt[:], red[:], befft[:])

                nc.sync.dma_start(out[b].rearrange("(p m) -> p m", p=128), outt[:])

    _split_multiwaits(nc)
    return nc


def _split_multiwaits(nc):
    """Walrus (neuronxcc codegen) fits only ONE sync-wait on compute-engine
    instruction structs.  Tile emits up to ~2 (engine self-sem + DMA lane).
    Hoist all but one wait onto same-engine InstDrain instructions placed
    immediately before the offender."""
    import concourse.mybir as mybir

    keep_multi = ("InstCall", "InstUnconditionalBranch", "InstISA",
                  "InstRegisterMove")
    for f in nc.m.functions:
        for blk in f.blocks:
            new = []
            changed = False
            for ins in blk.instructions:
                nm = type(ins).__name__
                si = getattr(ins, "sync_info", None)
                waits = list(si.on_wait) if si and si.on_wait else []
                if len(waits) > 1 and nm not in keep_multi:
                    for i, w in enumerate(waits[:-1]):
                        d = mybir.InstDrain(
                            name=f"{ins.name}-sw{i}", ins=[], outs=[]
                        )
                        d.engine = ins.engine
                        d.sync_info = mybir.SyncInfo(on_wait=[w], on_update=[])
                        new.append(d)
                    ins.sync_info = mybir.SyncInfo(
                        on_wait=[waits[-1]], on_update=list(si.on_update or [])
                    )
                    changed = True
                new.append(ins)
            if changed:
                blk.instructions = new


_BUILT = {}


def _get_nc(b_per=B_PER):
    if b_per not in _BUILT:
        _BUILT[b_per] = _build(b_per)
    return _BUILT[b_per]


def kernel(x, W0, b0, W1, W2):
    global LAST_RESULT
    x = np.asarray(x, dtype=np.float32).reshape(B, L)
    w0f = np.ascontiguousarray(np.asarray(W0, np.float32).reshape(-1))
    b0f = np.ascontiguousarray(np.asarray(b0, np.float32).reshape(-1))
    w1f = np.ascontiguousarray(np.asarray(W1, np.float32).reshape(-1))
    w2f = np.ascontiguousarray(np.asarray(W2, np.float32).reshape(-1))

    nc = _get_nc()
    in_maps = [
        {
            "x": np.ascontiguousarray(x[c * B_PER : (c + 1) * B_PER]),
            "w0": w0f,
            "b0": b0f,
            "w1": w1f,
            "w2": w2f,
        }
        for c in range(N_CORES)
    ]
    res = bass_utils.run_bass_kernel_spmd(
        nc, in_maps, core_ids=list(range(N_CORES)), trace=TRACE
    )
    LAST_RESULT = res
    out = np.concatenate([r["out"] for r in res.results], axis=0)
    return out.reshape(B, P_OUT, 1)
